# revision 5
# baseline (speedup 1.0000x reference)
# InternLM2-7B decode-step paged attention on 8 Trainium2 NeuronCores, v2.
#
# Sharding (tensor-parallel, per the source hooks):
#   - wqkv column-sharded: core c gets q heads 4c..4c+3 and kv head c
#   - wo row-sharded: core c gets rows for q heads 4c..4c+3
#   - KV cache sharded along the kv-head dim: core c gets head c
#   - output projection partials summed on the host (the all-reduce)
#
# v2 design (vs the v1 row-major kernel):
#   - scores computed directly TRANSPOSED: per (seq, l-chunk) the K-tile
#     [d=128, l=128] is the PE stationary operand and q [d, 4 heads] the
#     moving one, so psum holds S^T[l, (s,h)] with no row-scatter DMAs.
#   - softmax without max-subtraction (scores here are bounded ~|s|<=10,
#     exp stays in f32 range; softmax is shift-invariant) -> exp straight
#     from psum into bf16 attnT, already in the layout the V matmul needs.
#     A multiplicative 0/1 mask (mz) zeroes invalid cache positions.
#   - denominators via ones-vector stationary matmuls over attnT chunks;
#     normalization by a rank-1 replicate matmul of 1/sums, one DVE mul
#     per seq.
#   - V accumulated transposed too: V-chunk [l=128, d=128] stationary,
#     attn [l, 4] moving -> psum [d, 4] per seq; no output transposes.
#   - new token handled as a rank-1 (K=1) outer-product matmul appended
#     to each seq's V accumulation group.
import os
import sys

for _p in (
    "/opt/trn_rl_repo",
    "/root/.axon_site",
    "/root/.axon_site/_ro/trn_rl_repo",
    "/root/.axon_site/_ro/pypackages",
):
    if os.path.isdir(_p) and _p not in sys.path:
        sys.path.append(_p)

import numpy as np
import ml_dtypes

BF16NP = ml_dtypes.bfloat16

import concourse.bass as bass
from concourse import bacc
import concourse.mybir as mybir
import concourse.tile as tile
from concourse.masks import make_identity

B = 32          # batch (decoding sequences)
H = 32          # query heads
KVH = 8         # kv heads
G = 4           # query heads per kv head (= per core)
HD = 128        # head dim
D = 4096        # model dim
W = (G + 2) * HD  # per-core qkv shard width = 768
L = 4096        # kv positions per sequence
NCH = L // 128  # 32 l-chunks of 128
CGK = 2         # l-chunks per kT DMA tile / psum slab
CGN = NCH // CGK  # 16 chunk groups
VSG = 2         # seqs per v DMA tile
KT_ = D // 128  # 32 contraction tiles for the qkv projection
BLOCK = 64
NBLK = 64
NCORES = 8
THETA = 1e6
R = G * B       # 128 row-cols (s-major: col = 4*s + h)

F32 = mybir.dt.float32
BF16 = mybir.dt.bfloat16
SCALE = 1.0 / float(np.sqrt(HD))


def _emit(nc, tc, hT, wq, wo, kTg, vv, mz, cs, y):
    import contextlib

    Exp = mybir.ActivationFunctionType.Exp

    with contextlib.ExitStack() as ctx:
        singles = ctx.enter_context(tc.tile_pool(name="singles", bufs=1))
        wqp = ctx.enter_context(tc.tile_pool(name="wqp", bufs=2))
        ktp = ctx.enter_context(tc.tile_pool(name="ktp", bufs=4))
        vtp = ctx.enter_context(tc.tile_pool(name="vtp", bufs=3))
        stg = ctx.enter_context(tc.tile_pool(name="stg", bufs=3))
        # PSUM (8 banks): scp 3x1 + po 3x1 + sums 1 + rcps 1
        psA = ctx.enter_context(tc.tile_pool(name="psA", bufs=3, space="PSUM"))
        psB = ctx.enter_context(tc.tile_pool(name="psB", bufs=3, space="PSUM"))
        psD = ctx.enter_context(tc.tile_pool(name="psD", bufs=1, space="PSUM"))

        ident = singles.tile([128, 128], F32)
        make_identity(nc, ident)

        # ---- input loads (sync ring: hT/cs/wq/vv/y; scalar ring: mz/wo/kT)
        hT_sb = singles.tile([128, KT_, B], BF16)
        nc.sync.dma_start(hT_sb, hT)
        cs_sb = singles.tile([B, HD], F32)
        nc.sync.dma_start(cs_sb, cs)
        mz_sb = singles.tile([128, NCH, R], BF16)
        nc.scalar.dma_start(mz_sb, mz)
        wo_sb = singles.tile([128, G, D], BF16)
        nc.scalar.dma_start(wo_sb, wo.rearrange("(h p) n -> p h n", p=128))

        attnT = singles.tile([128, NCH, R], BF16)
        qT_buf = singles.tile([128, B, G], BF16)
        k_newT = singles.tile([128, B], F32)
        tmp_kq = singles.tile([128, B, G], F32)
        ones_bf = singles.tile([128, 1], BF16)
        nc.vector.memset(ones_bf, 1.0)
        ones_f = singles.tile([128, 1], F32)
        nc.vector.memset(ones_f, 1.0)
        ones_row = singles.tile([1, 128], F32)
        nc.vector.memset(ones_row, 1.0)
        pnew_row = singles.tile([1, R], BF16)
        vnew_row = singles.tile([1, B, HD], BF16)
        sums_f = singles.tile([1, R], F32)
        rc_row = singles.tile([1, R], F32)
        rc_sb = singles.tile([128, R], F32)
        outT_bf = singles.tile([128, B, G], BF16)

        # ---- fused QKV projection: qkv[B, W] = hT.T @ wq ----
        ps_q0 = psA.tile([128, 512], F32, tag="scp")
        ps_q1 = psA.tile([128, 512], F32, tag="scp")
        for tq in range(KT_ // 4):
            wt = wqp.tile([128, 4, W], BF16, tag="wt")
            nc.sync.dma_start(wt, wq[:, tq * 4 : (tq + 1) * 4, :])
            for u in range(4):
                t = tq * 4 + u
                nc.tensor.matmul(ps_q0[:B, 0:384], lhsT=hT_sb[:, t, :],
                                 rhs=wt[:, u, 0:384],
                                 start=(t == 0), stop=(t == KT_ - 1))
                nc.tensor.matmul(ps_q1[:B, 0:384], lhsT=hT_sb[:, t, :],
                                 rhs=wt[:, u, 384:W],
                                 start=(t == 0), stop=(t == KT_ - 1))
        qkv_sb = singles.tile([B, W], F32)
        nc.vector.tensor_copy(qkv_sb[:, 0:384], ps_q0[:B, 0:384])
        nc.vector.tensor_copy(qkv_sb[:, 384:W], ps_q1[:B, 0:384])

        # ---- RoPE on q (G heads) and k (1 head); v passthrough ----
        q_sb = singles.tile([B, G * HD], F32)
        k_sb = singles.tile([B, HD], F32)
        v_sb = singles.tile([B, HD], F32)
        nc.vector.tensor_copy(v_sb, qkv_sb[:, (G + 1) * HD : (G + 2) * HD])
        cosv = cs_sb[:, 0:64]
        sinv = cs_sb[:, 64:128]
        for j in range(G + 1):
            src = qkv_sb[:, j * HD : (j + 1) * HD]
            dst = q_sb[:, j * HD : (j + 1) * HD] if j < G else k_sb[:, :]
            a = src[:, 0:64]
            b = src[:, 64:128]
            t1 = stg.tile([B, 64], F32, tag="rt1")
            t2 = stg.tile([B, 64], F32, tag="rt2")
            nc.vector.tensor_mul(t1, a, cosv)
            nc.vector.tensor_mul(t2, b, sinv)
            nc.vector.tensor_sub(dst[:, 0:64], t1, t2)
            t3 = stg.tile([B, 64], F32, tag="rt1")
            t4 = stg.tile([B, 64], F32, tag="rt2")
            nc.vector.tensor_mul(t3, b, cosv)
            nc.vector.tensor_mul(t4, a, sinv)
            nc.vector.tensor_add(dst[:, 64:128], t3, t4)

        # ---- qT (pre-scaled, bf16): qT_buf[d, s, h]; k_newT[d, s] ----
        for h in range(G):
            ps_t = psA.tile([128, 512], F32, tag="scp")
            nc.tensor.transpose(ps_t[:, :B], q_sb[:, h * HD : (h + 1) * HD],
                                ident[:B, :B])
            nc.vector.tensor_scalar_mul(out=qT_buf[:, :, h], in0=ps_t[:, :B],
                                        scalar1=SCALE)
        ps_t = psA.tile([128, 512], F32, tag="scp")
        nc.tensor.transpose(ps_t[:, :B], k_sb[:, :], ident[:B, :B])
        nc.vector.tensor_copy(k_newT, ps_t[:, :B])

        # ---- new-token: p_new[1, (s,h)] = exp(qT . k_new); v_new row ----
        v_sbb = singles.tile([B, HD], BF16)
        nc.vector.tensor_copy(v_sbb, v_sb)
        nc.gpsimd.dma_start(vnew_row[0:1, :, :], v_sbb[:, :])
        for s in range(B):
            nc.vector.tensor_scalar_mul(out=tmp_kq[:, s, :],
                                        in0=qT_buf[:, s, :],
                                        scalar1=k_newT[:, s : s + 1])
        pnew_ps = psB.tile([128, 128], F32, tag="po")
        nc.tensor.matmul(pnew_ps[0:1, 0:R], lhsT=ones_f[:, 0:1],
                         rhs=tmp_kq[:, :, :], start=True, stop=True)
        nc.scalar.activation(out=pnew_row[0:1, :], in_=pnew_ps[0:1, 0:R],
                             func=Exp)

        # ---- scores: attnT[l, c, (s,h)] = exp(S^T) * mask; sums ----
        sums_ps = psD.tile([1, R], F32, tag="sums")
        for cg in range(CGN):
            kt = ktp.tile([128, B, CGK, 128], BF16, tag="kt")
            nc.sync.dma_start(kt, kTg[cg, :, :, :, :])
            scp = psA.tile([128, 512], F32, tag="scp")
            for s in range(B):
                for u in range(CGK):
                    o = u * 128 + 4 * s
                    nc.tensor.matmul(scp[:, o : o + 4], lhsT=kt[:, s, u, :],
                                     rhs=qT_buf[:, s, :],
                                     start=True, stop=True)
            att = attnT[:, CGK * cg : CGK * cg + CGK, :]
            nc.scalar.activation(out=att.rearrange("p u r -> p (u r)"),
                                 in_=scp[:, 0 : CGK * 128], func=Exp)
            nc.vector.tensor_mul(att, att,
                                 mz_sb[:, CGK * cg : CGK * cg + CGK, :])
            for u in range(CGK):
                c = CGK * cg + u
                nc.tensor.matmul(sums_ps[0:1, :], lhsT=ones_bf[:, 0:1],
                                 rhs=attnT[:, c, :],
                                 start=(c == 0), stop=(c == NCH - 1))

        # ---- 1/(sums + p_new), replicated to all partitions ----
        nc.vector.tensor_copy(sums_f, sums_ps[0:1, :])
        nc.vector.tensor_add(sums_f, sums_f, pnew_row[0:1, :])
        nc.vector.reciprocal(rc_row, sums_f)
        rc_ps = psD.tile([128, R], F32, tag="rcps")
        nc.tensor.matmul(rc_ps[:, :], lhsT=ones_row[0:1, :],
                         rhs=rc_row[0:1, :], start=True, stop=True)
        nc.vector.tensor_copy(rc_sb, rc_ps[:, :])

        # ---- V phase: outT[d, (s,h)] = (V^T @ attn + v_new x p_new) * rc
        for sg in range(B // VSG):
            vt = vtp.tile([128, VSG, NCH, HD], BF16, tag="vt")
            nc.sync.dma_start(vt, vv[sg, :, :, :, :])
            for sl in range(VSG):
                s = sg * VSG + sl
                ps_o = psB.tile([128, 128], F32, tag="po")
                for c in range(NCH):
                    nc.tensor.matmul(ps_o[:, 0:4], lhsT=vt[:, sl, c, :],
                                     rhs=attnT[:, c, 4 * s : 4 * s + 4],
                                     start=(c == 0), stop=False)
                nc.tensor.matmul(ps_o[:, 0:4], lhsT=vnew_row[0:1, s, :],
                                 rhs=pnew_row[0:1, 4 * s : 4 * s + 4],
                                 start=False, stop=True)
                nc.vector.tensor_mul(outT_bf[:, s, :], ps_o[:, 0:4],
                                     rc_sb[:, 4 * s : 4 * s + 4])

        # ---- output projection partial: y = outT.T @ wo_shard ----
        for n in range(D // 512):
            ps_y = psA.tile([128, 512], F32, tag="scp")
            for h in range(G):
                nc.tensor.matmul(ps_y[:B, :], lhsT=outT_bf[:, :, h],
                                 rhs=wo_sb[:, h, n * 512 : (n + 1) * 512],
                                 start=(h == 0), stop=(h == G - 1))
            yst = stg.tile([B, 512], F32, tag="yst")
            nc.any.tensor_copy(yst, ps_y[:B, :])
            nc.scalar.dma_start(y[:, n * 512 : (n + 1) * 512], yst)


_NC_CACHE = None


def build_bass():
    global _NC_CACHE
    if _NC_CACHE is not None:
        return _NC_CACHE
    nc = bacc.Bacc("TRN2")
    hT = nc.dram_tensor("hT", [128, KT_, B], BF16, kind="ExternalInput")
    wq = nc.dram_tensor("wq", [128, KT_, W], BF16, kind="ExternalInput")
    wo = nc.dram_tensor("wo", [G * HD, D], BF16, kind="ExternalInput")
    kTg = nc.dram_tensor("kTg", [CGN, 128, B, CGK, 128], BF16,
                         kind="ExternalInput")
    vv = nc.dram_tensor("vv", [B // VSG, 128, VSG, NCH, HD], BF16,
                        kind="ExternalInput")
    mz = nc.dram_tensor("mz", [128, NCH, R], BF16, kind="ExternalInput")
    cs = nc.dram_tensor("cs", [B, HD], F32, kind="ExternalInput")
    y = nc.dram_tensor("y", [B, D], F32, kind="ExternalOutput")
    with tile.TileContext(nc) as tc:
        _emit(nc, tc, hT[:, :, :], wq[:, :, :], wo[:, :], kTg[:, :, :, :, :],
              vv[:, :, :, :, :], mz[:, :, :], cs[:, :], y[:, :])
    nc.finalize()
    _NC_CACHE = nc
    return nc


def make_host_inputs(hidden_states, wqkv, wo, k_cache, v_cache,
                     position_ids_1d, block_offsets, kv_seqlens):
    """Shard + preprocess full inputs into 8 per-core in_maps."""
    hidden_states = np.asarray(hidden_states, dtype=np.float32)
    wqkv = np.asarray(wqkv, dtype=np.float32)
    wo = np.asarray(wo, dtype=np.float32)
    k_cache = np.asarray(k_cache, dtype=np.float32)
    v_cache = np.asarray(v_cache, dtype=np.float32)
    position_ids_1d = np.asarray(position_ids_1d, dtype=np.int32)
    block_offsets = np.asarray(block_offsets, dtype=np.int32)
    kv_seqlens = np.asarray(kv_seqlens, dtype=np.int32)

    hTd = np.ascontiguousarray(
        hidden_states.T.reshape(KT_, 128, B).transpose(1, 0, 2)
    ).astype(BF16NP)  # [128, KT_, B]

    # RoPE tables (f32, matching the reference convention)
    inv_freq = (1.0 / (THETA ** (np.arange(0, HD, 2, dtype=np.float64) / HD)))
    ang = position_ids_1d.astype(np.float64)[:, None] * inv_freq[None, :]
    cs_host = np.concatenate(
        [np.cos(ang), np.sin(ang)], axis=1).astype(np.float32)  # [B, 128]

    # validity: cache position j valid iff j < seqlen-1 (cache row at
    # seqlen-1 is replaced by the new token, handled separately)
    j = np.arange(L, dtype=np.int64)[None, :]
    valid = (j < (kv_seqlens.astype(np.int64)[:, None] - 1))  # [B, L] bool

    # multiplicative bf16 mask in attnT layout [p, c, 4s+h]
    validT = valid.reshape(B, NCH, 128).transpose(2, 1, 0)  # [p, c, s]
    mz_host = np.ascontiguousarray(
        np.repeat(validT.astype(np.float32), G, axis=2)).astype(BF16NP)

    # paged gather: per-sequence kv via block table (a permutation of blocks)
    ident_blocks = np.array_equal(block_offsets.ravel(),
                                  np.arange(B * NBLK, dtype=np.int64))

    kx = np.moveaxis(k_cache, 2, 0)  # [KVH, NUM_BLOCKS, BLOCK, HD] (view)
    vx = np.moveaxis(v_cache, 2, 0)

    vmaskf = valid.astype(np.float32)[:, :, None]  # [B, L, 1]

    in_maps = []
    for c in range(NCORES):
        if ident_blocks:
            kg = kx[c].reshape(B, L, HD)
            vg = vx[c].reshape(B, L, HD)
        else:
            kg = kx[c][block_offsets].reshape(B, L, HD)
            vg = vx[c][block_offsets].reshape(B, L, HD)
        # kTg[cg, d, s, u, l] = K[s, (CGK*cg+u)*128+l, d]
        kTg_c = np.ascontiguousarray(
            kg.reshape(B, CGN, CGK, 128, HD).transpose(1, 4, 0, 2, 3)
        ).astype(BF16NP)
        # vv[sg, p, sl, c, d] = V[sg*VSG+sl, c*128+p, d], invalid pos zeroed
        vm = vg * vmaskf
        vv_c = np.ascontiguousarray(
            vm.reshape(B // VSG, VSG, NCH, 128, HD).transpose(0, 3, 1, 2, 4)
        ).astype(BF16NP)
        # wq[p, t, w] = wq_full[t*128+p, w] (partition-major for big descs)
        wq_c = np.ascontiguousarray(np.concatenate([
            wqkv[:, c * G * HD : (c + 1) * G * HD],
            wqkv[:, H * HD + c * HD : H * HD + (c + 1) * HD],
            wqkv[:, (H + KVH) * HD + c * HD : (H + KVH) * HD + (c + 1) * HD],
        ], axis=1).reshape(KT_, 128, W).transpose(1, 0, 2)).astype(BF16NP)
        wo_c = np.ascontiguousarray(
            wo[c * G * HD : (c + 1) * G * HD, :]).astype(BF16NP)  # [G*HD, D]
        in_maps.append(dict(hT=hTd, wq=wq_c, wo=wo_c, kTg=kTg_c, vv=vv_c,
                            mz=mz_host, cs=cs_host))
    return in_maps


def kernel(**inputs):
    from concourse.bass_utils import run_bass_kernel_spmd

    in_maps = make_host_inputs(
        inputs["hidden_states"], inputs["wqkv"], inputs["wo"],
        inputs["k_cache"], inputs["v_cache"], inputs["position_ids_1d"],
        inputs["block_offsets"], inputs["kv_seqlens"])
    nc = build_bass()
    res = run_bass_kernel_spmd(nc, in_maps, core_ids=list(range(NCORES)))
    y = np.zeros((B, D), dtype=np.float32)
    for r in res.results:
        y += np.asarray(r["y"], dtype=np.float32)
    return y
